# revision 16
# baseline (speedup 1.0000x reference)
"""Trainium2 Bass kernel for nn_Attention_module_52166672777937.

Length-aware chunk-packed attention, data-parallel over batch on 8 cores.

Only the attention output at the LAST valid position of each sequence is
consumed (take_along_axis with lengths-1) and attention is causal, so per
sequence only ONE query row and the key/value positions 0..len-1 matter.
The baseline exploited the single query; this version additionally skips
all work past each sequence's length:

  * positions are processed in 128-wide CHUNKS; sequence b needs only
    ceil(len_b/128) chunks instead of LP/128 = 8.
  * the 32 sequences are LPT-assigned to the 8 cores (4 seqs each) to
    equalize total chunk counts; every core runs the same static program
    of C = max(core totals) chunks (padded with inert chunks).
  * all owner-dependence (which sequence a chunk belongs to, its causal
    boundary, its positional-encoding rows) lives in per-core packed DMA
    data: the packed char stream, packed pe slices, packed score masks.
    The SPMD program itself is core-uniform.
  * scores for all 4 sequences' 32 head-columns are computed per chunk;
    the packed mask (-1e30 on non-owner columns and beyond-boundary rows)
    zeroes foreign contributions after exp, so softmax denominators and
    ctx accumulate over the whole chunk stream into single [32,.]-shaped
    PSUM groups.
  * q-prep and the prediction head are batched over the 4 sequences
    (large-moving matmuls instead of per-sequence slivers).
  * a dummy-matmul warmup chain keeps the PE HAM clock-gate warm through
    the initial DMA ramp; DMAs are ordered group-major so the chunk loop
    never starves.

The kernel is JIT-specialized to the actual lengths at first call (C is
derived from the inputs, the module is cached by C).
"""

import math

import ml_dtypes
import numpy as np
import sys

sys.path.insert(0, "/opt/trn_rl_repo")

import concourse.bacc as bacc
import concourse.bass as bass
import concourse.mybir as mybir
import concourse.tile as tile
from concourse.bass_utils import run_bass_kernel_spmd

dt = mybir.dt
AF = mybir.ActivationFunctionType
ALU = mybir.AluOpType
PSUM = bass.MemorySpace.PSUM

N_CORES = 8
B, L = 32, 1000
CH = 128                  # chunk width (positions)
NCH = 256                 # vocabulary
E = 512                   # embedding dim
D = 512                   # d_model
NH, DH = 8, 64            # heads
HS = 512                  # pred hidden size
NOUT = 8
BPC = B // N_CORES        # sequences per core
NSC = BPC * NH            # score columns (4 seqs x 8 heads)
NEG = -1.0e30
SCALE = 1.0 / math.sqrt(DH)
SENT = 300.0              # padding sentinel char (bf16-exact, not in vocab)
N_WARM = 14               # dummy warmup matmuls during the DMA ramp

# fmix column layout: cvals 2 | hmask 4*8 | pelT 4*4 | hm32 4*32 | id4 4
FM_CV = 0
FM_HM = 2
FM_PL = FM_HM + 4 * NH
FM_H32 = FM_PL + 4 * BPC
FM_ID4 = FM_H32 + 4 * NSC
FM_W = FM_ID4 + 4


def _group_widths(C):
    gw = [4] * (C // 4)
    if C % 4:
        gw.append(C % 4)
    return gw


def _build(C, b1v, b2v):
    gw = _group_widths(C)
    G = len(gw)
    goff = [0]
    for w in gw:
        goff.append(goff[-1] + w)

    nc = bacc.Bacc(
        "TRN2", target_bir_lowering=False, debug=False, num_devices=N_CORES
    )

    f32 = dt.float32
    bf16 = dt.bfloat16
    f8 = dt.float8e4
    DR = mybir.MatmulPerfMode.DoubleRow

    d_drow = nc.dram_tensor("drow", [1, C * CH + BPC], bf16,
                            kind="ExternalInput")
    d_emb = nc.dram_tensor("emb", [NCH, E], dt.float8e4,
                           kind="ExternalInput")
    d_wb = nc.dram_tensor("wb", [E, 3 * D], bf16, kind="ExternalInput")
    d_wr = nc.dram_tensor("wr", [D, HS], bf16, kind="ExternalInput")
    d_w2 = nc.dram_tensor("w2", [HS, NOUT], bf16, kind="ExternalInput")
    d_pe = nc.dram_tensor("pe", [128, C * 4 * CH], bf16, kind="ExternalInput")
    d_mask = nc.dram_tensor("mask", [128, C * NSC], bf16,
                            kind="ExternalInput")
    d_fmix = nc.dram_tensor("fmix", [128, FM_W], f32, kind="ExternalInput")
    d_id32 = nc.dram_tensor("id32", [32, 32], bf16, kind="ExternalInput")
    d_out = nc.dram_tensor("out", [BPC, 1], f32, kind="ExternalOutput")

    with tile.TileContext(nc) as tc:
        with (
            tc.tile_pool(name="const", bufs=1) as cp,
            tc.tile_pool(name="work", bufs=2) as wp,
            tc.tile_pool(name="psx", bufs=2, space=PSUM) as psx,
            tc.tile_pool(name="psv", bufs=2, space=PSUM) as psv,
            tc.tile_pool(name="pss", bufs=2, space=PSUM) as pss,
            tc.tile_pool(name="psc", bufs=1, space=PSUM) as psc,
            tc.tile_pool(name="psd", bufs=1, space=PSUM) as psd,
        ):
            # ---------------- warmup (no DMA dependency) -----------------
            ones128 = cp.tile([128, 1], bf16, name="ones128", tag="ones128")
            nc.vector.memset(ones128[:], 1.0)
            wub = cp.tile([128, 512], bf16, name="wub", tag="wub")
            nc.vector.memset(wub[:], 0.25)
            for wi in range(N_WARM):
                p = pss.tile([1, 512], f32, name=f"wu{wi}", tag="sp")
                nc.tensor.matmul(p[:], ones128[:], wub[:])

            # ---------------- constant DMAs (ordered by first use) -------
            fmix_sb = cp.tile([128, FM_W], f32, name="fmix", tag="fmix")
            nc.sync.dma_start(out=fmix_sb[:], in_=d_fmix[:])
            cvals = fmix_sb[:, FM_CV:FM_CV + 2]
            hmask = [fmix_sb[:, FM_HM + 8 * m:FM_HM + 8 * (m + 1)]
                     for m in range(4)]
            pelT = [fmix_sb[:, FM_PL + 4 * m:FM_PL + 4 * (m + 1)]
                    for m in range(4)]
            hm32 = [fmix_sb[:, FM_H32 + NSC * m:FM_H32 + NSC * (m + 1)]
                    for m in range(4)]
            id4f = fmix_sb[0:4, FM_ID4:FM_ID4 + 4]

            idxl_sb = cp.tile([128, BPC], bf16, name="idxl", tag="idxl")
            nc.sync.dma_start(
                out=idxl_sb[:],
                in_=d_drow[:, C * CH:].to_broadcast((128, BPC)),
            )
            drow_sb = cp.tile([128, C * CH], bf16, name="drow", tag="drow")
            embp_sb = cp.tile([128, 2, E], f8, name="embp", tag="embp")
            pe_sb = [cp.tile([128, 4 * gw[g] * CH], bf16, name=f"pe{g}",
                             tag=f"pe{g}") for g in range(G)]
            mask_sb = cp.tile([128, C * NSC], bf16, name="mask", tag="mask")

            def dma_drow(g):
                nc.sync.dma_start(
                    out=drow_sb[:, goff[g] * CH:goff[g + 1] * CH],
                    in_=d_drow[:, goff[g] * CH:goff[g + 1] * CH].to_broadcast(
                        (128, gw[g] * CH)),
                )

            def dma_pe(g):
                nc.sync.dma_start(
                    out=pe_sb[g][:],
                    in_=d_pe[:, 4 * CH * goff[g]:4 * CH * goff[g + 1]],
                )

            def dma_mask(g):
                nc.sync.dma_start(
                    out=mask_sb[:, goff[g] * NSC:goff[g + 1] * NSC],
                    in_=d_mask[:, goff[g] * NSC:goff[g + 1] * NSC],
                )

            dma_drow(0)
            nc.sync.dma_start(
                out=embp_sb[:],
                in_=d_emb[:].rearrange("(c p) n -> p c n", p=128),
            )
            dma_pe(0)

            wqp_sb = cp.tile([128, 4, D], bf16, name="wqp", tag="wqp")
            nc.sync.dma_start(
                out=wqp_sb[:],
                in_=d_wb[:, 0:D].rearrange("(c p) n -> p c n", p=128),
            )
            wqT_sb = [wqp_sb[:, e, :] for e in range(4)]

            id32_sb = cp.tile([32, 32], bf16, name="id32", tag="id32")
            nc.sync.dma_start(out=id32_sb[:], in_=d_id32[:])

            if G > 1:
                dma_drow(1)
                dma_pe(1)
            wkp_sb = cp.tile([128, 4, E], bf16, name="wkp", tag="wkp")
            nc.sync.dma_start(
                out=wkp_sb[:],
                in_=d_wb[:, D:2 * D].rearrange("(c p) n -> p c n", p=128),
            )
            wk_sb = [wkp_sb[:, c, :] for c in range(4)]
            dma_mask(0)
            if G > 1:
                dma_mask(1)

            if G > 2:
                dma_drow(2)
                dma_pe(2)
            wvp_sb = cp.tile([128, 4, D], bf16, name="wvp", tag="wvp")
            nc.sync.dma_start(
                out=wvp_sb[:],
                in_=d_wb[:, 2 * D:3 * D].rearrange("(c p) n -> p c n", p=128),
            )
            wvT_sb = [wvp_sb[:, e, :] for e in range(4)]
            if G > 2:
                dma_mask(2)

            for g in range(3, G):
                dma_drow(g)
                dma_pe(g)
                dma_mask(g)

            wr_sb = cp.tile([128, 4, HS], bf16, name="wr", tag="wr")
            nc.sync.dma_start(
                out=wr_sb[:], in_=d_wr[:].rearrange("(c p) n -> p c n", p=128)
            )
            w1T_sb = [wr_sb[:, m, :] for m in range(4)]
            w2p_sb = cp.tile([128, 4, NOUT], bf16, name="w2p", tag="w2p")
            nc.sync.dma_start(
                out=w2p_sb[:],
                in_=d_w2[:].rearrange("(c p) n -> p c n", p=128),
            )
            w2T_sb = [w2p_sb[:, m, :] for m in range(4)]

            # ---------------- gather helpers ----------------------------
            xT_sb = [[cp.tile([128, gw[g] * CH], bf16, name=f"xT{g}_{m}",
                              tag=f"xT{g}_{m}") for m in range(4)]
                     for g in range(G)]

            def emit_oh(g):
                oh = wp.tile([128, 2, gw[g] * CH], f8, name=f"oh{g}",
                             tag="oh", bufs=3)
                for c in range(2):
                    nc.vector.tensor_scalar(
                        oh[:, c, :], drow_sb[:, goff[g] * CH:goff[g + 1] * CH],
                        cvals[:, c:c + 1], None, ALU.is_equal,
                    )
                return oh

            def emit_gather_block(g, m, oh):
                # gather e-block m of group g and evict (+pe) immediately;
                # fp8 DoubleRow contracts both 128-char planes in one pass
                p = psx.tile([128, gw[g] * CH], f32, name=f"xtp{g}_{m}",
                             tag="xtp")
                nc.tensor.matmul(
                    p[:], embp_sb[:, :, m * 128:(m + 1) * 128], oh[:],
                    perf_mode=DR,
                )
                w = gw[g] * CH
                nc.vector.tensor_tensor(
                    xT_sb[g][m][:], p[:],
                    pe_sb[g][:, m * w:(m + 1) * w], ALU.add,
                )

            # ---------------- prologue ----------------------------------
            # x_last gather -> q (batched over the 4 sequences)
            ohl = cp.tile([128, 2, BPC], f8, name="ohl", tag="ohl")
            for c in range(2):
                nc.vector.tensor_scalar(
                    ohl[:, c, :], idxl_sb[:], cvals[:, c:c + 1], None,
                    ALU.is_equal
                )
            oh0 = emit_oh(0)
            xlast_sb = cp.tile([128, 4, BPC], bf16, name="xlast", tag="xlast")
            for m in range(4):
                p = pss.tile([128, BPC], f32, name=f"xlp{m}", tag="sp")
                nc.tensor.matmul(
                    p[:], embp_sb[:, :, m * 128:(m + 1) * 128], ohl[:],
                    perf_mode=DR,
                )
                nc.vector.tensor_tensor(
                    xlast_sb[:, m, :], p[:], pelT[m], ALU.add
                )
            # gather group 0 early (needs only drow+emb+pe0 DMAs)
            for m in range(4):
                emit_gather_block(0, m, oh0)
            # q_all [4, 512] = x_last.T @ WqT   (bq is asserted zero)
            qp = psv.tile([BPC, D], f32, name="qp", tag="vp")
            for m in range(4):
                nc.tensor.matmul(
                    qp[:], xlast_sb[:, m, :], wqT_sb[m][:],
                    start=(m == 0), stop=(m == 3),
                )
            q_sb = cp.tile([BPC, D], f32, name="q_sb", tag="q_sb")
            nc.scalar.copy(q_sb[:], qp[:])
            # qT [128, 4(db), 4(s)]
            qT_sb = cp.tile([128, 4, BPC], f32, name="qT", tag="qT")
            for db in range(4):
                tp = pss.tile([128, BPC], f32, name=f"qTp{db}", tag="sp")
                nc.tensor.transpose(
                    tp[:], q_sb[:, db * 128:(db + 1) * 128], id4f
                )
                nc.vector.tensor_copy(qT_sb[:, db, :], tp[:])
            # group 1 gather
            if G > 1:
                oh1 = emit_oh(1)
                for m in range(4):
                    emit_gather_block(1, m, oh1)
            # qblk [128, 4(db), 32]: per (db, s) hmask * qT scalar column
            qblk_sb = cp.tile([128, 4, NSC], bf16, name="qblk", tag="qblk")
            for db in range(4):
                for s in range(BPC):
                    nc.vector.tensor_scalar(
                        qblk_sb[:, db, s * NH:(s + 1) * NH], hmask[db],
                        qT_sb[:, db, s:s + 1], None, ALU.mult,
                    )
            # qkv_all [32, 512e] = qblk.T @ Wk
            qkvp = psv.tile([NSC, E], f32, name="qkvp", tag="vp")
            for db in range(4):
                nc.tensor.matmul(
                    qkvp[:], qblk_sb[:, db, :], wk_sb[db][:],
                    start=(db == 0), stop=(db == 3),
                )
            qkv_sb = cp.tile([NSC, E], bf16, name="qkv_sb", tag="qkv_sb")
            nc.scalar.copy(qkv_sb[:], qkvp[:])
            # qkvT [128, 4(m), 32]
            qkvT_sb = cp.tile([128, 4, NSC], bf16, name="qkvT", tag="qkvT")
            for m in range(4):
                tp = pss.tile([128, NSC], bf16, name=f"qkvTp{m}", tag="sp")
                nc.tensor.transpose(
                    tp[:], qkv_sb[:, m * 128:(m + 1) * 128], id32_sb[:]
                )
                nc.vector.tensor_copy(qkvT_sb[:, m, :], tp[:])

            # ---------------- chunk loop --------------------------------
            ctxp = psc.tile([NSC, D], f32, name="ctxp", tag="cp")
            dnTp = psd.tile([NSC, 1], f32, name="dnTp", tag="dn")

            pend_ctx = []  # (chunk_idx, aT, v) awaiting ctx/dn emission

            def emit_ctx_dn(force=False):
                while pend_ctx and (force or len(pend_ctx) > 1):
                    i, aT, v = pend_ctx.pop(0)
                    nc.tensor.matmul(
                        ctxp[:], aT[:], v[:],
                        start=(i == 0), stop=(i == C - 1),
                    )
                    nc.tensor.matmul(
                        dnTp[:], aT[:], ones128[:],
                        start=(i == 0), stop=(i == C - 1),
                    )

            for g in range(G):
                # software-pipelined gather of group g+2, spread over cycle
                gl = g + 2
                if gl < G:
                    ohn = emit_oh(gl)
                    gq = [m for m in range(4)]
                else:
                    ohn, gq = None, []
                for j in range(gw[g]):
                    npop = (((j + 1) * 4 + gw[g] - 1) // gw[g]
                            - (j * 4 + gw[g] - 1) // gw[g]) if gq else 0
                    for _ in range(min(npop, len(gq))):
                        emit_gather_block(gl, gq.pop(0), ohn)
                    i = goff[g] + j
                    # scores + V share the xT stationary
                    slp = pss.tile([128, NSC], f32, name=f"slp{i}", tag="sp")
                    vp = psv.tile([128, D], f32, name=f"vp{i}", tag="vp")
                    for m in range(4):
                        stat = xT_sb[g][m][:, j * CH:(j + 1) * CH]
                        nc.tensor.matmul(
                            vp[:], stat, wvT_sb[m][:],
                            start=(m == 0), stop=(m == 3),
                        )
                        nc.tensor.matmul(
                            slp[:], stat, qkvT_sb[:, m, :],
                            start=(m == 0), stop=(m == 3),
                        )
                    slpm = wp.tile([128, NSC], f32, name=f"slpm{i}",
                                   tag="slpm", bufs=3)
                    nc.vector.tensor_tensor(
                        slpm[:], slp[:],
                        mask_sb[:, i * NSC:(i + 1) * NSC], ALU.add,
                    )
                    aT = wp.tile([128, NSC], bf16, name=f"aT{i}", tag="aT",
                                 bufs=4)
                    nc.scalar.activation(aT[:], slpm[:], AF.Exp, scale=SCALE)
                    v = wp.tile([128, D], bf16, name=f"v{i}", tag="v", bufs=3)
                    nc.scalar.copy(v[:], vp[:])
                    pend_ctx.append((i, aT, v))
                    emit_ctx_dn()
                while gq:
                    emit_gather_block(gl, gq.pop(0), ohn)
            emit_ctx_dn(force=True)

            # ---------------- softmax normalize + ctx.T ------------------
            rec = wp.tile([NSC, 1], f32, name="rec", tag="rec")
            nc.vector.reciprocal(rec[:], dnTp[:])
            ctx_sb = cp.tile([NSC, D], bf16, name="ctx_sb", tag="ctx_sb")
            nc.vector.tensor_scalar(
                ctx_sb[:], ctxp[:], rec[:], None, ALU.mult
            )
            # ctxT4 [128, 4(db), 4(s)]: transpose blocks, head-select, reduce
            ctxT4 = cp.tile([128, 4, BPC], bf16, name="ctxT4", tag="ctxT4")
            for db in range(4):
                tp = pss.tile([128, NSC], bf16, name=f"ctp{db}", tag="sp")
                nc.tensor.transpose(
                    tp[:], ctx_sb[:, db * 128:(db + 1) * 128], id32_sb[:]
                )
                scr = wp.tile([128, BPC, NH], f32, name=f"scr{db}", tag="scr")
                nc.vector.tensor_tensor(scr[:], tp[:], hm32[db], ALU.mult)
                with nc.allow_low_precision("fp32 accum, bf16 round"):
                    nc.vector.tensor_reduce(
                        ctxT4[:, db, :], scr[:], mybir.AxisListType.X, ALU.add
                    )

            # ------------- prediction head (batched, transposed) ---------
            # hT4 [128(hs), 4(hb), 4(s)] = W1 @ ctx_last, computed block-wise
            hT4p = psv.tile([128, 4, BPC], f32, name="hT4p", tag="vp")
            for hb in range(4):
                for db in range(4):
                    nc.tensor.matmul(
                        hT4p[:, hb, :],
                        w1T_sb[db][:, hb * 128:(hb + 1) * 128],
                        ctxT4[:, db, :],
                        start=(db == 0), stop=(db == 3),
                    )
            ht1 = wp.tile([128, 4, BPC], f32, name="ht1", tag="ht1")
            nc.vector.tensor_scalar(ht1[:], hT4p[:], b1v, None, ALU.add)
            hT_sb = cp.tile([128, 4, BPC], bf16, name="hT", tag="hT")
            nc.vector.scalar_tensor_tensor(
                hT_sb[:], ht1[:], 0.01, ht1[:], ALU.mult, ALU.max
            )
            r2p = pss.tile([BPC, NOUT], f32, name="r2p", tag="sp")
            for hb in range(4):
                nc.tensor.matmul(
                    r2p[:], hT_sb[:, hb, :], w2T_sb[hb][:],
                    start=(hb == 0), stop=(hb == 3),
                )
            r2r = wp.tile([BPC, NOUT], f32, name="r2r", tag="r2r")
            nc.vector.tensor_scalar(r2r[:], r2p[:], b2v, 0.0, ALU.add,
                                    ALU.max)
            mt = wp.tile([BPC, 1], f32, name="mt", tag="mt")
            nc.vector.tensor_reduce(
                mt[:], r2r[:], mybir.AxisListType.X, ALU.add
            )
            ms = wp.tile([BPC, 1], f32, name="ms", tag="ms")
            nc.vector.tensor_scalar(ms[:], mt[:], 1.0 / NOUT, None, ALU.mult)
            out_sb = cp.tile([BPC, 1], f32, name="out_sb", tag="out_sb")
            nc.vector.scalar_tensor_tensor(
                out_sb[:], ms[:], 0.01, ms[:], ALU.mult, ALU.max
            )
            nc.sync.dma_start(out=d_out[:], in_=out_sb[:])

    nc.compile()
    return nc


_CACHE = {}


def _get_module(C, b1v=0.01, b2v=0.01):
    key = (C, b1v, b2v)
    if key not in _CACHE:
        _CACHE[key] = _build(C, b1v, b2v)
    return _CACHE[key]


def _pos_encoding(n):
    pos = np.arange(n, dtype=np.float32)[:, None]
    div = np.exp(
        np.arange(0, D, 2, dtype=np.float32) * (-math.log(10000.0) / D)
    )
    pe = np.zeros((n, D), np.float32)
    pe[:, 0::2] = np.sin(pos * div)
    pe[:, 1::2] = np.cos(pos * div)
    return pe


def _plan(lengths):
    """LPT-balance sequences into 8 groups of 4 by chunk count."""
    nch = -(-lengths // CH)  # ceil
    order = np.argsort(-nch, kind="stable")
    groups = [[] for _ in range(N_CORES)]
    sums = [0] * N_CORES
    for idx in order:
        cands = [g for g in range(N_CORES) if len(groups[g]) < BPC]
        g = min(cands, key=lambda g: sums[g])
        groups[g].append(int(idx))
        sums[g] += int(nch[idx])
    return groups, int(max(sums)), nch


def make_in_maps(data, lengths, emb, Wq, bq, Wk, bk, Wv, bv, W1, b1, W2, b2):
    # the kernel folds the K projection into the score contraction and
    # skips the q/v biases entirely; all three are zero for this module.
    assert float(np.abs(np.asarray(bq)).max()) == 0.0
    assert float(np.abs(np.asarray(bk)).max()) == 0.0
    assert float(np.abs(np.asarray(bv)).max()) == 0.0
    # b1/b2 are uniform fills; they enter as scalar constants.
    b1 = np.asarray(b1)
    b2 = np.asarray(b2)
    assert float(np.abs(b1 - b1.flat[0]).max()) == 0.0
    assert float(np.abs(b2 - b2.flat[0]).max()) == 0.0

    data = np.asarray(data)
    lengths = np.asarray(lengths).astype(np.int64)
    groups, C, nch = _plan(lengths)
    gwl = _group_widths(C)
    p = lengths - 1

    bfl = ml_dtypes.bfloat16
    pe = _pos_encoding(1024)                       # [1024, D]

    wb = np.concatenate(
        [np.asarray(Wq).T, np.asarray(Wk), np.asarray(Wv).T], axis=1
    ).astype(bfl)                                  # [512, 1536]
    wr = np.ascontiguousarray(np.asarray(W1).T, dtype=bfl)   # [D, HS]
    w2t = np.ascontiguousarray(np.asarray(W2).T, dtype=bfl)  # [HS, 8]

    # fmix: cvals | hmask | pelT | hm32 | id4 (pelT filled per-core below)
    fmix0 = np.zeros((128, FM_W), np.float32)
    fmix0[:, FM_CV + 0] = np.arange(128)
    fmix0[:, FM_CV + 1] = np.arange(128, 256)
    rows = np.arange(128)
    for db in range(4):
        hm = np.zeros((128, NH), np.float32)
        hm[rows, 2 * db + rows // 64] = 1.0
        fmix0[:, FM_HM + 8 * db:FM_HM + 8 * (db + 1)] = hm
        fmix0[:, FM_H32 + NSC * db:FM_H32 + NSC * (db + 1)] = np.tile(
            hm, (1, BPC)
        )
    fmix0[0:4, FM_ID4:FM_ID4 + 4] = np.eye(4)

    dpad = np.full((B, 1024), SENT, np.float32)
    dpad[:, :L] = data[:, :L]

    shared = {
        "emb": np.ascontiguousarray(emb, dtype=ml_dtypes.float8_e4m3),
        "wb": np.ascontiguousarray(wb),
        "wr": wr,
        "w2": w2t,
        "id32": np.eye(32, dtype=bfl),
    }
    in_maps = []
    for c in range(N_CORES):
        seqs = groups[c]
        # chunk list: (owner_slot, k) in sequence-major order + pads
        chunks = []
        for s, b in enumerate(seqs):
            chunks += [(s, k) for k in range(int(nch[b]))]
        chunks += [(-1, 0)] * (C - len(chunks))

        drow = np.full(C * CH + BPC, SENT, np.float32)
        mask = np.full((128, C, NSC), NEG, np.float32)
        # pe packed per group as [128, 4(m), gw(j), CH]
        pe_parts = []
        gi0 = 0
        for w in gwl:
            arr = np.zeros((128, 4, w, CH), np.float32)
            for j in range(w):
                i = gi0 + j
                s, k = chunks[i]
                if s >= 0:
                    pe_blk = pe[k * CH:(k + 1) * CH, :]   # [128 l, 512 e]
                    arr[:, :, j, :] = pe_blk.T.reshape(
                        4, 128, CH).transpose(1, 0, 2)
            pe_parts.append(arr.reshape(128, 4 * w * CH))
            gi0 += w
        for i, (s, k) in enumerate(chunks):
            if s < 0:
                continue
            b = seqs[s]
            drow[i * CH:(i + 1) * CH] = dpad[b, k * CH:(k + 1) * CH]
            lpos = k * CH + np.arange(CH)
            valid = lpos <= p[b]                          # [128]
            mask[:, i, s * NH:(s + 1) * NH] = np.where(
                valid[:, None], 0.0, NEG
            )
        drow[C * CH:] = data[np.arange(B), p][seqs]

        fmix = fmix0.copy()
        pl = pe[p[seqs], :]                               # [4, 512]
        for m in range(4):
            fmix[:, FM_PL + 4 * m:FM_PL + 4 * (m + 1)] = (
                pl[:, m * 128:(m + 1) * 128].T
            )

        m = dict(shared)
        m["drow"] = drow.reshape(1, -1).astype(bfl)
        m["pe"] = np.ascontiguousarray(
            np.concatenate(pe_parts, axis=1)).astype(bfl)
        m["mask"] = np.ascontiguousarray(
            mask.reshape(128, C * NSC)).astype(bfl)
        m["fmix"] = np.ascontiguousarray(fmix)
        in_maps.append(m)
    return in_maps, groups, C


def kernel(data, lengths, emb, Wq, bq, Wk, bk, Wv, bv, W1, b1, W2, b2):
    in_maps, groups, C = make_in_maps(
        data, lengths, emb, Wq, bq, Wk, bk, Wv, bv, W1, b1, W2, b2
    )
    nc = _get_module(C, float(np.asarray(b1).flat[0]),
                     float(np.asarray(b2).flat[0]))
    res = run_bass_kernel_spmd(nc, in_maps, list(range(N_CORES)))
    out = np.zeros(B, np.float32)
    for c in range(N_CORES):
        vals = res.results[c]["out"].reshape(BPC)
        for s, b in enumerate(groups[c]):
            out[b] = vals[s]
    return out


# revision 17
# speedup vs baseline: 1.0364x; 1.0364x over previous
"""Trainium2 Bass kernel for nn_Attention_module_52166672777937.

Length-aware chunk-packed attention, data-parallel over batch on 8 cores.

Only the attention output at the LAST valid position of each sequence is
consumed (take_along_axis with lengths-1) and attention is causal, so per
sequence only ONE query row and the key/value positions 0..len-1 matter.
The baseline exploited the single query; this version additionally skips
all work past each sequence's length:

  * positions are processed in 128-wide CHUNKS; sequence b needs only
    ceil(len_b/128) chunks instead of LP/128 = 8.
  * the 32 sequences are LPT-assigned to the 8 cores (4 seqs each) to
    equalize total chunk counts; every core runs the same static program
    of C = max(core totals) chunks (padded with inert chunks).
  * all owner-dependence (which sequence a chunk belongs to, its causal
    boundary, its positional-encoding rows) lives in per-core packed DMA
    data: the packed char stream, packed pe slices, packed score masks.
    The SPMD program itself is core-uniform.
  * scores for all 4 sequences' 32 head-columns are computed per chunk;
    the packed mask (-1e30 on non-owner columns and beyond-boundary rows)
    zeroes foreign contributions after exp, so softmax denominators and
    ctx accumulate over the whole chunk stream into single [32,.]-shaped
    PSUM groups.
  * q-prep and the prediction head are batched over the 4 sequences
    (large-moving matmuls instead of per-sequence slivers).
  * a dummy-matmul warmup chain keeps the PE HAM clock-gate warm through
    the initial DMA ramp; DMAs are ordered group-major so the chunk loop
    never starves.

The kernel is JIT-specialized to the actual lengths at first call (C is
derived from the inputs, the module is cached by C).
"""

import math

import ml_dtypes
import numpy as np
import sys

sys.path.insert(0, "/opt/trn_rl_repo")

import concourse.bacc as bacc
import concourse.bass as bass
import concourse.mybir as mybir
import concourse.tile as tile
from concourse.bass_utils import run_bass_kernel_spmd

dt = mybir.dt
AF = mybir.ActivationFunctionType
ALU = mybir.AluOpType
PSUM = bass.MemorySpace.PSUM

N_CORES = 8
B, L = 32, 1000
CH = 128                  # chunk width (positions)
NCH = 256                 # vocabulary
E = 512                   # embedding dim
D = 512                   # d_model
NH, DH = 8, 64            # heads
HS = 512                  # pred hidden size
NOUT = 8
BPC = B // N_CORES        # sequences per core
NSC = BPC * NH            # score columns (4 seqs x 8 heads)
NEG = -1.0e30
SCALE = 1.0 / math.sqrt(DH)
SENT = 300.0              # padding sentinel char (bf16-exact, not in vocab)
N_WARM = 14               # dummy warmup matmuls during the DMA ramp

# fmix column layout: cvals 2 | hmask 4*8 | pelT 4*4 | hm32 4*32 | id4 4
FM_CV = 0
FM_HM = 2
FM_PL = FM_HM + 4 * NH
FM_H32 = FM_PL + 4 * BPC
FM_ID4 = FM_H32 + 4 * NSC
FM_SEL = FM_ID4 + 4
FM_W = FM_SEL + NSC


def _group_widths(C):
    gw = [4] * (C // 4)
    if C % 4:
        gw.append(C % 4)
    return gw


def _build(C, b1v, b2v):
    gw = _group_widths(C)
    G = len(gw)
    goff = [0]
    for w in gw:
        goff.append(goff[-1] + w)

    nc = bacc.Bacc(
        "TRN2", target_bir_lowering=False, debug=False, num_devices=N_CORES
    )

    f32 = dt.float32
    bf16 = dt.bfloat16
    f8 = dt.float8e4
    DR = mybir.MatmulPerfMode.DoubleRow

    d_drow = nc.dram_tensor("drow", [1, C * CH + BPC], bf16,
                            kind="ExternalInput")
    d_emb = nc.dram_tensor("emb", [NCH, E], dt.float8e4,
                           kind="ExternalInput")
    d_wb = nc.dram_tensor("wb", [E, 3 * D], bf16, kind="ExternalInput")
    d_wr = nc.dram_tensor("wr", [D, HS], bf16, kind="ExternalInput")
    d_w2 = nc.dram_tensor("w2", [HS, NOUT], bf16, kind="ExternalInput")
    d_pe = nc.dram_tensor("pe", [128, C * 4 * CH], bf16, kind="ExternalInput")
    d_mask = nc.dram_tensor("mask", [128, C * NSC], bf16,
                            kind="ExternalInput")
    d_fmix = nc.dram_tensor("fmix", [128, FM_W], f32, kind="ExternalInput")
    d_id32 = nc.dram_tensor("id32", [32, 32], bf16, kind="ExternalInput")
    d_out = nc.dram_tensor("out", [BPC, 1], f32, kind="ExternalOutput")

    with tile.TileContext(nc) as tc:
        with (
            tc.tile_pool(name="const", bufs=1) as cp,
            tc.tile_pool(name="work", bufs=2) as wp,
            tc.tile_pool(name="psx", bufs=2, space=PSUM) as psx,
            tc.tile_pool(name="psv", bufs=2, space=PSUM) as psv,
            tc.tile_pool(name="pss", bufs=2, space=PSUM) as pss,
            tc.tile_pool(name="psc", bufs=1, space=PSUM) as psc,
            tc.tile_pool(name="psd", bufs=1, space=PSUM) as psd,
        ):
            # ---------------- warmup (no DMA dependency) -----------------
            ones128 = cp.tile([128, 1], bf16, name="ones128", tag="ones128")
            nc.vector.memset(ones128[:], 1.0)
            wub = cp.tile([128, 512], bf16, name="wub", tag="wub")
            nc.vector.memset(wub[:], 0.25)
            for wi in range(N_WARM):
                p = pss.tile([1, 512], f32, name=f"wu{wi}", tag="sp")
                nc.tensor.matmul(p[:], ones128[:], wub[:])

            # ---------------- constant DMAs (ordered by first use) -------
            fmix_sb = cp.tile([128, FM_W], f32, name="fmix", tag="fmix")
            nc.sync.dma_start(out=fmix_sb[:], in_=d_fmix[:])
            cvals = fmix_sb[:, FM_CV:FM_CV + 2]
            hmask = [fmix_sb[:, FM_HM + 8 * m:FM_HM + 8 * (m + 1)]
                     for m in range(4)]
            pelT = [fmix_sb[:, FM_PL + 4 * m:FM_PL + 4 * (m + 1)]
                    for m in range(4)]
            hm32 = [fmix_sb[:, FM_H32 + NSC * m:FM_H32 + NSC * (m + 1)]
                    for m in range(4)]
            id4f = fmix_sb[0:4, FM_ID4:FM_ID4 + 4]
            self_sel = fmix_sb[:, FM_SEL:FM_SEL + NSC]

            idxl_sb = cp.tile([128, BPC], bf16, name="idxl", tag="idxl")
            nc.sync.dma_start(
                out=idxl_sb[:],
                in_=d_drow[:, C * CH:].to_broadcast((128, BPC)),
            )
            drow_sb = cp.tile([128, C * CH], bf16, name="drow", tag="drow")
            embp_sb = cp.tile([128, 2, E], f8, name="embp", tag="embp")
            pe_sb01 = cp.tile([128, 4 * CH * (goff[min(2, G)])], bf16,
                              name="pe01", tag="pe01")
            pe_sb2 = (cp.tile([128, 4 * CH * gw[2]], bf16, name="pe2",
                              tag="pe2") if G > 2 else None)
            pe_sb3p = (cp.tile([128, 4 * CH * (goff[G] - goff[3])], bf16,
                               name="pe3p", tag="pe3p") if G > 3 else None)
            pe_sb = []
            for g in range(G):
                if g < 2:
                    base = 4 * CH * goff[g]
                    pe_sb.append(pe_sb01[:, base:base + 4 * CH * gw[g]])
                elif g == 2:
                    pe_sb.append(pe_sb2[:])
                else:
                    base = 4 * CH * (goff[g] - goff[3])
                    pe_sb.append(pe_sb3p[:, base:base + 4 * CH * gw[g]])
            mask_sb = cp.tile([128, C * NSC], bf16, name="mask", tag="mask")

            nc.sync.dma_start(
                out=drow_sb[:],
                in_=d_drow[:, 0:C * CH].to_broadcast((128, C * CH)),
            )
            nc.sync.dma_start(
                out=embp_sb[:],
                in_=d_emb[:].rearrange("(c p) n -> p c n", p=128),
            )
            nc.sync.dma_start(
                out=pe_sb01[:],
                in_=d_pe[:, 0:4 * CH * goff[min(2, G)]],
            )

            wqp_sb = cp.tile([128, 4, D], bf16, name="wqp", tag="wqp")
            nc.sync.dma_start(
                out=wqp_sb[:],
                in_=d_wb[:, 0:D].rearrange("(c p) n -> p c n", p=128),
            )
            wqT_sb = [wqp_sb[:, e, :] for e in range(4)]

            id32_sb = cp.tile([32, 32], bf16, name="id32", tag="id32")
            nc.sync.dma_start(out=id32_sb[:], in_=d_id32[:])

            wkp_sb = cp.tile([128, 4, E], bf16, name="wkp", tag="wkp")
            nc.sync.dma_start(
                out=wkp_sb[:],
                in_=d_wb[:, D:2 * D].rearrange("(c p) n -> p c n", p=128),
            )
            wk_sb = [wkp_sb[:, c, :] for c in range(4)]
            nc.sync.dma_start(out=mask_sb[:], in_=d_mask[:])
            wvp_sb = cp.tile([128, 4, D], bf16, name="wvp", tag="wvp")
            nc.sync.dma_start(
                out=wvp_sb[:],
                in_=d_wb[:, 2 * D:3 * D].rearrange("(c p) n -> p c n", p=128),
            )
            wvT_sb = [wvp_sb[:, e, :] for e in range(4)]
            if G > 2:
                nc.sync.dma_start(
                    out=pe_sb2[:],
                    in_=d_pe[:, 4 * CH * goff[2]:4 * CH * goff[3]],
                )
            if G > 3:
                nc.sync.dma_start(
                    out=pe_sb3p[:],
                    in_=d_pe[:, 4 * CH * goff[3]:4 * CH * goff[G]],
                )

            wr_sb = cp.tile([128, 4, HS], bf16, name="wr", tag="wr")
            nc.sync.dma_start(
                out=wr_sb[:], in_=d_wr[:].rearrange("(c p) n -> p c n", p=128)
            )
            w1T_sb = [wr_sb[:, m, :] for m in range(4)]
            w2p_sb = cp.tile([128, 4, NOUT], bf16, name="w2p", tag="w2p")
            nc.sync.dma_start(
                out=w2p_sb[:],
                in_=d_w2[:].rearrange("(c p) n -> p c n", p=128),
            )
            w2T_sb = [w2p_sb[:, m, :] for m in range(4)]

            # ---------------- gather helpers ----------------------------
            xT_sb = [[cp.tile([128, gw[g] * CH], bf16, name=f"xT{g}_{m}",
                              tag=f"xT{g}_{m}") for m in range(4)]
                     for g in range(G)]

            def emit_oh(g):
                oh = wp.tile([128, 2, gw[g] * CH], f8, name=f"oh{g}",
                             tag="oh", bufs=3)
                for c in range(2):
                    nc.vector.tensor_scalar(
                        oh[:, c, :], drow_sb[:, goff[g] * CH:goff[g + 1] * CH],
                        cvals[:, c:c + 1], None, ALU.is_equal,
                    )
                return oh

            def emit_gather_block(g, m, oh):
                # gather e-block m of group g and evict (+pe) immediately;
                # fp8 DoubleRow contracts both 128-char planes in one pass
                p = psx.tile([128, gw[g] * CH], f32, name=f"xtp{g}_{m}",
                             tag="xtp")
                nc.tensor.matmul(
                    p[:], embp_sb[:, :, m * 128:(m + 1) * 128], oh[:],
                    perf_mode=DR,
                )
                w = gw[g] * CH
                pv = pe_sb[g]
                nc.vector.tensor_tensor(
                    xT_sb[g][m][:], p[:],
                    pv[:, m * w:(m + 1) * w], ALU.add,
                )

            # ---------------- prologue ----------------------------------
            # x_last gather -> q (batched over the 4 sequences)
            ohl = cp.tile([128, 2, BPC], f8, name="ohl", tag="ohl")
            for c in range(2):
                nc.vector.tensor_scalar(
                    ohl[:, c, :], idxl_sb[:], cvals[:, c:c + 1], None,
                    ALU.is_equal
                )
            oh0 = emit_oh(0)
            xlast_sb = cp.tile([128, 4, BPC], bf16, name="xlast", tag="xlast")
            for m in range(4):
                p = pss.tile([128, BPC], f32, name=f"xlp{m}", tag="sp")
                nc.tensor.matmul(
                    p[:], embp_sb[:, :, m * 128:(m + 1) * 128], ohl[:],
                    perf_mode=DR,
                )
                nc.vector.tensor_tensor(
                    xlast_sb[:, m, :], p[:], pelT[m], ALU.add
                )
            # gather group 0 early (needs only drow+emb+pe0 DMAs)
            for m in range(4):
                emit_gather_block(0, m, oh0)
            # q_all [4, 512] = x_last.T @ WqT   (bq is asserted zero)
            qp = psv.tile([BPC, D], f32, name="qp", tag="vp")
            for m in range(4):
                nc.tensor.matmul(
                    qp[:], xlast_sb[:, m, :], wqT_sb[m][:],
                    start=(m == 0), stop=(m == 3),
                )
            q_sb = cp.tile([BPC, D], f32, name="q_sb", tag="q_sb")
            nc.scalar.copy(q_sb[:], qp[:])
            # qT [128, 4(db), 4(s)]
            qT_sb = cp.tile([128, 4, BPC], f32, name="qT", tag="qT")
            for db in range(4):
                tp = pss.tile([128, BPC], f32, name=f"qTp{db}", tag="sp")
                nc.tensor.transpose(
                    tp[:], q_sb[:, db * 128:(db + 1) * 128], id4f
                )
                nc.vector.tensor_copy(qT_sb[:, db, :], tp[:])
            # group 1 gather
            if G > 1:
                oh1 = emit_oh(1)
                for m in range(4):
                    emit_gather_block(1, m, oh1)
            # qblk [128, 4(db), 32]: per (db, s) hmask * qT scalar column
            qblk_sb = cp.tile([128, 4, NSC], bf16, name="qblk", tag="qblk")
            for db in range(4):
                for s in range(BPC):
                    nc.vector.tensor_scalar(
                        qblk_sb[:, db, s * NH:(s + 1) * NH], hmask[db],
                        qT_sb[:, db, s:s + 1], None, ALU.mult,
                    )
            # qkv_all [32, 512e] = qblk.T @ Wk
            qkvp = psv.tile([NSC, E], f32, name="qkvp", tag="vp")
            for db in range(4):
                nc.tensor.matmul(
                    qkvp[:], qblk_sb[:, db, :], wk_sb[db][:],
                    start=(db == 0), stop=(db == 3),
                )
            qkv_sb = cp.tile([NSC, E], bf16, name="qkv_sb", tag="qkv_sb")
            nc.scalar.copy(qkv_sb[:], qkvp[:])
            # qkvT [128, 4(m), 32]
            qkvT_sb = cp.tile([128, 4, NSC], bf16, name="qkvT", tag="qkvT")
            for m in range(4):
                tp = pss.tile([128, NSC], bf16, name=f"qkvTp{m}", tag="sp")
                nc.tensor.transpose(
                    tp[:], qkv_sb[:, m * 128:(m + 1) * 128], id32_sb[:]
                )
                nc.vector.tensor_copy(qkvT_sb[:, m, :], tp[:])

            # ---------------- chunk loop --------------------------------
            ctxp = psc.tile([NSC, D], f32, name="ctxp", tag="cp")
            dnall = psd.tile([128, 1], f32, name="dnall", tag="dn")

            pend_ctx = []  # (chunk_idx, aT slice, v) awaiting ctx emission

            def emit_ctx_dn(force=False):
                while pend_ctx and (force or len(pend_ctx) > 1):
                    i, aT, v = pend_ctx.pop(0)
                    nc.tensor.matmul(
                        ctxp[:], aT, v[:],
                        start=(i == 0), stop=(i == C - 1),
                    )

            for g in range(G):
                # software-pipelined gather of group g+2, spread over cycle
                gl = g + 2
                if gl < G:
                    ohn = emit_oh(gl)
                    gq = [m for m in range(4)]
                else:
                    ohn, gq = None, []
                aT_g = wp.tile([128, gw[g], NSC], bf16, name=f"aTg{g}",
                               tag="aT", bufs=3)
                for j in range(gw[g]):
                    npop = (((j + 1) * 4 + gw[g] - 1) // gw[g]
                            - (j * 4 + gw[g] - 1) // gw[g]) if gq else 0
                    for _ in range(min(npop, len(gq))):
                        emit_gather_block(gl, gq.pop(0), ohn)
                    i = goff[g] + j
                    # scores + V share the xT stationary
                    slp = pss.tile([128, NSC], f32, name=f"slp{i}", tag="sp")
                    vp = psv.tile([128, D], f32, name=f"vp{i}", tag="vp")
                    for m in range(4):
                        stat = xT_sb[g][m][:, j * CH:(j + 1) * CH]
                        nc.tensor.matmul(
                            vp[:], stat, wvT_sb[m][:],
                            start=(m == 0), stop=(m == 3),
                        )
                        nc.tensor.matmul(
                            slp[:], stat, qkvT_sb[:, m, :],
                            start=(m == 0), stop=(m == 3),
                        )
                    slpm = wp.tile([128, NSC], f32, name=f"slpm{i}",
                                   tag="slpm", bufs=3)
                    nc.vector.tensor_tensor(
                        slpm[:], slp[:],
                        mask_sb[:, i * NSC:(i + 1) * NSC], ALU.add,
                    )
                    nc.scalar.activation(aT_g[:, j, :], slpm[:], AF.Exp,
                                         scale=SCALE)
                    v = wp.tile([128, D], bf16, name=f"v{i}", tag="v", bufs=3)
                    nc.scalar.copy(v[:], vp[:])
                    pend_ctx.append((i, aT_g[:, j, :], v))
                    emit_ctx_dn()
                while gq:
                    emit_gather_block(gl, gq.pop(0), ohn)
                # per-group softmax-denominator partial: [gw*32, 1] rows
                nc.tensor.matmul(
                    dnall[0:gw[g] * NSC, :], aT_g[:, :, :], ones128[:],
                    start=(g == 0), stop=(g == G - 1),
                )
            emit_ctx_dn(force=True)

            # ---------------- softmax normalize + ctx.T ------------------
            dnall_sb = wp.tile([128, 1], f32, name="dnall_sb",
                                tag="dnall_sb")
            nc.vector.tensor_copy(dnall_sb[:], dnall[:])
            dnT = pss.tile([NSC, 1], f32, name="dnT", tag="sp")
            nc.tensor.matmul(dnT[:], self_sel, dnall_sb[:])
            rec = wp.tile([NSC, 1], f32, name="rec", tag="rec")
            nc.vector.reciprocal(rec[:], dnT[:])
            ctx_sb = cp.tile([NSC, D], bf16, name="ctx_sb", tag="ctx_sb")
            nc.vector.tensor_scalar(
                ctx_sb[:], ctxp[:], rec[:], None, ALU.mult
            )
            # ctxT4 [128, 4(db), 4(s)]: transpose blocks, head-select, reduce
            ctxT4 = cp.tile([128, 4, BPC], bf16, name="ctxT4", tag="ctxT4")
            for db in range(4):
                tp = pss.tile([128, NSC], bf16, name=f"ctp{db}", tag="sp")
                nc.tensor.transpose(
                    tp[:], ctx_sb[:, db * 128:(db + 1) * 128], id32_sb[:]
                )
                scr = wp.tile([128, BPC, NH], f32, name=f"scr{db}", tag="scr")
                nc.vector.tensor_tensor(scr[:], tp[:], hm32[db], ALU.mult)
                with nc.allow_low_precision("fp32 accum, bf16 round"):
                    nc.vector.tensor_reduce(
                        ctxT4[:, db, :], scr[:], mybir.AxisListType.X, ALU.add
                    )

            # ------------- prediction head (batched, transposed) ---------
            # hT4 [128(hs), 4(hb), 4(s)] = W1 @ ctx_last, computed block-wise
            hT4p = psv.tile([128, 4, BPC], f32, name="hT4p", tag="vp")
            for hb in range(4):
                for db in range(4):
                    nc.tensor.matmul(
                        hT4p[:, hb, :],
                        w1T_sb[db][:, hb * 128:(hb + 1) * 128],
                        ctxT4[:, db, :],
                        start=(db == 0), stop=(db == 3),
                    )
            ht1 = wp.tile([128, 4, BPC], f32, name="ht1", tag="ht1")
            nc.vector.tensor_scalar(ht1[:], hT4p[:], b1v, None, ALU.add)
            hT_sb = cp.tile([128, 4, BPC], bf16, name="hT", tag="hT")
            nc.vector.scalar_tensor_tensor(
                hT_sb[:], ht1[:], 0.01, ht1[:], ALU.mult, ALU.max
            )
            r2p = pss.tile([BPC, NOUT], f32, name="r2p", tag="sp")
            for hb in range(4):
                nc.tensor.matmul(
                    r2p[:], hT_sb[:, hb, :], w2T_sb[hb][:],
                    start=(hb == 0), stop=(hb == 3),
                )
            r2r = wp.tile([BPC, NOUT], f32, name="r2r", tag="r2r")
            nc.vector.tensor_scalar(r2r[:], r2p[:], b2v, 0.0, ALU.add,
                                    ALU.max)
            mt = wp.tile([BPC, 1], f32, name="mt", tag="mt")
            nc.vector.tensor_reduce(
                mt[:], r2r[:], mybir.AxisListType.X, ALU.add
            )
            ms = wp.tile([BPC, 1], f32, name="ms", tag="ms")
            nc.vector.tensor_scalar(ms[:], mt[:], 1.0 / NOUT, None, ALU.mult)
            out_sb = cp.tile([BPC, 1], f32, name="out_sb", tag="out_sb")
            nc.vector.scalar_tensor_tensor(
                out_sb[:], ms[:], 0.01, ms[:], ALU.mult, ALU.max
            )
            nc.sync.dma_start(out=d_out[:], in_=out_sb[:])

    nc.compile()
    return nc


_CACHE = {}


def _get_module(C, b1v=0.01, b2v=0.01):
    key = (C, b1v, b2v)
    if key not in _CACHE:
        _CACHE[key] = _build(C, b1v, b2v)
    return _CACHE[key]


def _pos_encoding(n):
    pos = np.arange(n, dtype=np.float32)[:, None]
    div = np.exp(
        np.arange(0, D, 2, dtype=np.float32) * (-math.log(10000.0) / D)
    )
    pe = np.zeros((n, D), np.float32)
    pe[:, 0::2] = np.sin(pos * div)
    pe[:, 1::2] = np.cos(pos * div)
    return pe


def _plan(lengths):
    """LPT-balance sequences into 8 groups of 4 by chunk count."""
    nch = -(-lengths // CH)  # ceil
    order = np.argsort(-nch, kind="stable")
    groups = [[] for _ in range(N_CORES)]
    sums = [0] * N_CORES
    for idx in order:
        cands = [g for g in range(N_CORES) if len(groups[g]) < BPC]
        g = min(cands, key=lambda g: sums[g])
        groups[g].append(int(idx))
        sums[g] += int(nch[idx])
    return groups, int(max(sums)), nch


def make_in_maps(data, lengths, emb, Wq, bq, Wk, bk, Wv, bv, W1, b1, W2, b2):
    # the kernel folds the K projection into the score contraction and
    # skips the q/v biases entirely; all three are zero for this module.
    assert float(np.abs(np.asarray(bq)).max()) == 0.0
    assert float(np.abs(np.asarray(bk)).max()) == 0.0
    assert float(np.abs(np.asarray(bv)).max()) == 0.0
    # b1/b2 are uniform fills; they enter as scalar constants.
    b1 = np.asarray(b1)
    b2 = np.asarray(b2)
    assert float(np.abs(b1 - b1.flat[0]).max()) == 0.0
    assert float(np.abs(b2 - b2.flat[0]).max()) == 0.0

    data = np.asarray(data)
    lengths = np.asarray(lengths).astype(np.int64)
    groups, C, nch = _plan(lengths)
    gwl = _group_widths(C)
    p = lengths - 1

    bfl = ml_dtypes.bfloat16
    pe = _pos_encoding(1024)                       # [1024, D]

    wb = np.concatenate(
        [np.asarray(Wq).T, np.asarray(Wk), np.asarray(Wv).T], axis=1
    ).astype(bfl)                                  # [512, 1536]
    wr = np.ascontiguousarray(np.asarray(W1).T, dtype=bfl)   # [D, HS]
    w2t = np.ascontiguousarray(np.asarray(W2).T, dtype=bfl)  # [HS, 8]

    # fmix: cvals | hmask | pelT | hm32 | id4 (pelT filled per-core below)
    fmix0 = np.zeros((128, FM_W), np.float32)
    fmix0[:, FM_CV + 0] = np.arange(128)
    fmix0[:, FM_CV + 1] = np.arange(128, 256)
    rows = np.arange(128)
    for db in range(4):
        hm = np.zeros((128, NH), np.float32)
        hm[rows, 2 * db + rows // 64] = 1.0
        fmix0[:, FM_HM + 8 * db:FM_HM + 8 * (db + 1)] = hm
        fmix0[:, FM_H32 + NSC * db:FM_H32 + NSC * (db + 1)] = np.tile(
            hm, (1, BPC)
        )
    fmix0[0:4, FM_ID4:FM_ID4 + 4] = np.eye(4)
    fmix0[:, FM_SEL:FM_SEL + NSC] = np.tile(np.eye(NSC), (BPC, 1))

    dpad = np.full((B, 1024), SENT, np.float32)
    dpad[:, :L] = data[:, :L]

    shared = {
        "emb": np.ascontiguousarray(emb, dtype=ml_dtypes.float8_e4m3),
        "wb": np.ascontiguousarray(wb),
        "wr": wr,
        "w2": w2t,
        "id32": np.eye(32, dtype=bfl),
    }
    in_maps = []
    for c in range(N_CORES):
        seqs = groups[c]
        # chunk list: (owner_slot, k) in sequence-major order + pads
        chunks = []
        for s, b in enumerate(seqs):
            chunks += [(s, k) for k in range(int(nch[b]))]
        chunks += [(-1, 0)] * (C - len(chunks))

        drow = np.full(C * CH + BPC, SENT, np.float32)
        mask = np.full((128, C, NSC), NEG, np.float32)
        # pe packed per group as [128, 4(m), gw(j), CH]
        pe_parts = []
        gi0 = 0
        for w in gwl:
            arr = np.zeros((128, 4, w, CH), np.float32)
            for j in range(w):
                i = gi0 + j
                s, k = chunks[i]
                if s >= 0:
                    pe_blk = pe[k * CH:(k + 1) * CH, :]   # [128 l, 512 e]
                    arr[:, :, j, :] = pe_blk.T.reshape(
                        4, 128, CH).transpose(1, 0, 2)
            pe_parts.append(arr.reshape(128, 4 * w * CH))
            gi0 += w
        for i, (s, k) in enumerate(chunks):
            if s < 0:
                continue
            b = seqs[s]
            drow[i * CH:(i + 1) * CH] = dpad[b, k * CH:(k + 1) * CH]
            lpos = k * CH + np.arange(CH)
            valid = lpos <= p[b]                          # [128]
            mask[:, i, s * NH:(s + 1) * NH] = np.where(
                valid[:, None], 0.0, NEG
            )
        drow[C * CH:] = data[np.arange(B), p][seqs]

        fmix = fmix0.copy()
        pl = pe[p[seqs], :]                               # [4, 512]
        for m in range(4):
            fmix[:, FM_PL + 4 * m:FM_PL + 4 * (m + 1)] = (
                pl[:, m * 128:(m + 1) * 128].T
            )

        m = dict(shared)
        m["drow"] = drow.reshape(1, -1).astype(bfl)
        m["pe"] = np.ascontiguousarray(
            np.concatenate(pe_parts, axis=1)).astype(bfl)
        m["mask"] = np.ascontiguousarray(
            mask.reshape(128, C * NSC)).astype(bfl)
        m["fmix"] = np.ascontiguousarray(fmix)
        in_maps.append(m)
    return in_maps, groups, C


def kernel(data, lengths, emb, Wq, bq, Wk, bk, Wv, bv, W1, b1, W2, b2):
    in_maps, groups, C = make_in_maps(
        data, lengths, emb, Wq, bq, Wk, bk, Wv, bv, W1, b1, W2, b2
    )
    nc = _get_module(C, float(np.asarray(b1).flat[0]),
                     float(np.asarray(b2).flat[0]))
    res = run_bass_kernel_spmd(nc, in_maps, list(range(N_CORES)))
    out = np.zeros(B, np.float32)
    for c in range(N_CORES):
        vals = res.results[c]["out"].reshape(BPC)
        for s, b in enumerate(groups[c]):
            out[b] = vals[s]
    return out
